# revision 22
# baseline (speedup 1.0000x reference)
"""CrossEntropyLossWithGaussianSmoothedLabels on 8 TRN2 NeuronCores.

Math: the reference's scatter-built smoothed label at class j is exactly
w[|j-t|] for |j-t|<=3 (w = [1, e^-.5, e^-1, e^-2]), clamped writes always
being overwritten by the nearer-distance write. So

  loss = mean_r( W_r * logsumexp(x_r) - sum_o w[|o|] * x_r[t_r+o] )

with W_r = sum of valid window weights. The gather term is computed on the
TensorEngine without any per-row gather:

  sum_r sum_o w[o] x[r, t_r+o] = sum_{|m-n|<=3} (H^T X)[m, n] * w[n-m]

where H is the one-hot target matrix. H^T X is accumulated in PSUM via 6
banded 128x128 fp16 matmuls per 128-row tile (overlapping class blocks with
ownership-deduped band masks). logsumexp runs max-free (|x| < 6) via the
ScalarEngine's fused exp+accumulate. Each core emits per-row W*lse and 6
per-partition band partials; the host sums (the unshard step) in float64.
"""

import math
from contextlib import ExitStack

import numpy as np

import concourse.bacc as bacc
import concourse.tile as tile
from concourse import mybir
from concourse.bass_utils import run_bass_kernel_spmd

P = 128
C = 722
NCORES = 8
ROWS = 16 * 2048
RPC = ROWS // NCORES  # 4096 rows per core
KPM = 4               # row-tiles per macro tile (per DMA)
NB = 6
BLK = [0, 124, 248, 372, 496, 594]  # even bases -> 4B-aligned fp16 slices
URANGES = [(0, 124), (124, 248), (248, 372), (372, 496), (496, 594), (594, 722)]
WDEC = [1.0, math.exp(-0.5), math.exp(-1.0), math.exp(-2.0)]

f32 = mybir.dt.float32
f16 = mybir.dt.float16
i32 = mybir.dt.int32


def _band_masks() -> np.ndarray:
    """[128, 6*128] f32: block-local band weights, each global band entry
    owned by exactly one block (by min(m,n) ownership range)."""
    m = np.zeros((P, NB * P), np.float32)
    for b in range(NB):
        s = BLK[b]
        lo, hi = URANGES[b]
        for i in range(P):
            for o in range(-3, 4):
                j = i + o
                if 0 <= j < P:
                    mg, ng = s + i, s + j
                    if mg < C and ng < C and lo <= min(mg, ng) < hi:
                        m[i, b * P + j] = WDEC[abs(o)]
    return m


def _build(rpc: int, stage: int = 6):
    nt = rpc // P
    nm = nt // KPM
    assert nt % KPM == 0
    nc = bacc.Bacc(
        "TRN2", target_bir_lowering=False, debug=False, num_devices=NCORES
    )
    AF = mybir.ActivationFunctionType
    OP = mybir.AluOpType

    pred = nc.dram_tensor("prediction", [rpc, C], f32, kind="ExternalInput").ap()
    tgt = nc.dram_tensor("target_pair", [P, rpc // P, 2], i32, kind="ExternalInput").ap()
    band = nc.dram_tensor("band", [P, NB * P], f32, kind="ExternalInput").ap()
    out = nc.dram_tensor("out", [P, nt + NB], f32, kind="ExternalOutput").ap()

    with tile.TileContext(nc) as tc, ExitStack() as ctx:
        xp = ctx.enter_context(tc.tile_pool(name="x", bufs=5))
        hp = ctx.enter_context(tc.tile_pool(name="h", bufs=4))
        cp = ctx.enter_context(tc.tile_pool(name="xc", bufs=4))
        sp = ctx.enter_context(tc.tile_pool(name="scr", bufs=2))
        sg = ctx.enter_context(tc.tile_pool(name="singles", bufs=1))
        pp = ctx.enter_context(tc.tile_pool(name="psum", bufs=1, space="PSUM"))

        # warm the Exp/Ln ACT table set at t~0 so the first real exp
        # doesn't eat the ~2.6us table load
        warm = sg.tile([P, 1], f32)
        nc.vector.memset(warm[:], 0.0)
        nc.scalar.activation(out=warm[:], in_=warm[:], func=AF.Exp)

        # row r lives at partition r // nt, column r % nt: every DMA line is
        # per-partition contiguous (target: 64 i32; prediction: 2888B rows)
        t_pair = sg.tile([P, nt, 2], i32)
        nc.sync.dma_start(out=t_pair[:], in_=tgt)

        # macro-batched loads: KPM row tiles per DMA, alternating queues
        xmac = {}

        def load_macro(m):
            xm = xp.tile([P, KPM, C], f32, name=f"xm{m}", tag="x")
            eng = nc.sync if m % 2 == 0 else nc.gpsimd
            src_ap = pred[m * KPM * P:(m + 1) * KPM * P, :].rearrange(
                "(k p) c -> p k c", p=P)
            eng.dma_start(out=xm[:], in_=src_ap)
            xmac[m] = xm

        nprefetch_mac = 3
        for m in range(min(nprefetch_mac, nt // KPM)):
            load_macro(m)

        band_sb = sg.tile([P, NB * P], f32)
        nc.sync.dma_start(out=band_sb[:], in_=band)
        outsb = sg.tile([P, nt + NB], f32)
        nc.vector.memset(outsb[:], 0.0)

        iota_i = sg.tile([P, C], i32)
        if stage >= 4:
            nc.gpsimd.iota(iota_i[:], pattern=[[1, C]], base=0, channel_multiplier=0)
        iota_h = sg.tile([P, C], f16)
        if stage >= 4:
            nc.vector.tensor_copy(out=iota_h[:], in_=iota_i[:])

        # t_f[p, i] = float(target[i*128 + p]) from the low int32 words
        t_f = sg.tile([P, nt], f32)
        if stage >= 3:
            nc.vector.tensor_copy(out=t_f[:], in_=t_pair[:, :, 0])

        sumexp = sg.tile([P, nt], f32)
        wr = sg.tile([P, nt], f32)

        psum_blk = [pp.tile([P, P], f32, name=f"psumblk{b}", tag=f"psum{b}") for b in range(NB)]

        # W_r = 1 + sum_d w_d*([t>=d] + [t<=721-d])
        nc.vector.memset(wr[:], 1.0)
        for d in (1, 2, 3) if stage >= 3 else ():
            tmp = sp.tile([P, nt], f32, tag="wtmp")
            nc.vector.tensor_scalar(
                out=tmp[:], in0=t_f[:], scalar1=d - 0.5, scalar2=WDEC[d],
                op0=OP.is_ge, op1=OP.mult,
            )
            nc.vector.tensor_tensor(out=wr[:], in0=wr[:], in1=tmp[:], op=OP.add)
            tmp2 = sp.tile([P, nt], f32, tag="wtmp")
            nc.vector.tensor_scalar(
                out=tmp2[:], in0=t_f[:], scalar1=(C - 1 - d) + 0.5, scalar2=WDEC[d],
                op0=OP.is_le, op1=OP.mult,
            )
            nc.vector.tensor_tensor(out=wr[:], in0=wr[:], in1=tmp2[:], op=OP.add)

        for i in range(nt) if stage >= 2 else ():
            m, k = divmod(i, KPM)
            if m not in xmac:
                load_macro(m)
            x = xmac[m][:, k, :]

            xh = cp.tile([P, C], f16, tag="xh")
            if stage >= 5:
                nc.vector.tensor_copy(out=xh[:], in_=x)

            esc = sp.tile([P, C], f16, tag="esc")
            nc.scalar.activation(
                out=esc[:], in_=x, func=AF.Exp,
                accum_out=sumexp[:, i:i + 1],
            )
            h = hp.tile([P, C], f16)
            if stage >= 4:
                nc.vector.tensor_scalar(
                    out=h[:], in0=iota_h[:], scalar1=t_f[:, i:i + 1],
                    scalar2=None, op0=OP.is_equal, op1=OP.bypass,
                )
            for b in range(NB) if stage >= 5 else ():
                s = BLK[b]
                nc.tensor.matmul(
                    psum_blk[b][:], h[:, s:s + P], xh[:, s:s + P],
                    start=(i == 0), stop=(i == nt - 1),
                )

        lse = sg.tile([P, nt], f32)
        if stage >= 2:
            nc.scalar.activation(out=lse[:], in_=sumexp[:], func=AF.Ln)
        if stage >= 3:
            nc.vector.tensor_tensor(out=outsb[:, 0:nt], in0=wr[:], in1=lse[:], op=OP.mult)
        elif stage >= 2:
            nc.vector.tensor_copy(out=outsb[:, 0:nt], in_=lse[:])
        for b in range(NB) if stage >= 6 else ():
            mscr = sp.tile([P, P], f32, tag="mscr")
            nc.vector.tensor_tensor(
                out=mscr[:], in0=psum_blk[b][:],
                in1=band_sb[:, b * P:(b + 1) * P], op=OP.mult,
            )
            nc.vector.tensor_reduce(
                out=outsb[:, nt + b:nt + b + 1], in_=mscr[:],
                axis=mybir.AxisListType.X, op=OP.add,
            )
        nc.sync.dma_start(out=out, in_=outsb[:])

    nc.compile()
    return nc


def _build_raw(rpc: int):
    """Raw-bacc version: explicit engine programs + semaphores, no Tile
    framework preamble/teardown."""
    nt = rpc // P
    nc = bacc.Bacc(
        "TRN2", target_bir_lowering=False, debug=False, num_devices=NCORES
    )
    AF = mybir.ActivationFunctionType
    OP = mybir.AluOpType

    pred = nc.dram_tensor("prediction", [rpc, C], f32, kind="ExternalInput").ap()
    tgt = nc.dram_tensor("target_pair", [P, nt, 2], i32, kind="ExternalInput").ap()
    band = nc.dram_tensor("band", [P, NB * P], f32, kind="ExternalInput").ap()
    out = nc.dram_tensor("out", [P, nt + NB], f32, kind="ExternalOutput").ap()

    XB = 14   # x ring depth
    HR = 4    # xh/h ring depth

    x_ring = nc.alloc_sbuf_tensor("x_ring", [P, XB, C], f32).ap()
    xh_ring = nc.alloc_sbuf_tensor("xh_ring", [P, HR, C], f16).ap()
    h_ring = nc.alloc_sbuf_tensor("h_ring", [P, HR, C], f16).ap()
    esc = nc.alloc_sbuf_tensor("esc", [P, C], f16).ap()
    iota_i = nc.alloc_sbuf_tensor("iota_i", [P, C], i32).ap()
    iota_h = nc.alloc_sbuf_tensor("iota_h", [P, C], f16).ap()
    t_pair = nc.alloc_sbuf_tensor("t_pair", [P, nt, 2], i32).ap()
    t_f = nc.alloc_sbuf_tensor("t_f", [P, nt], f32).ap()
    band_sb = nc.alloc_sbuf_tensor("band_sb", [P, NB * P], f32).ap()
    sumexp = nc.alloc_sbuf_tensor("sumexp", [P, nt], f32).ap()
    lse = nc.alloc_sbuf_tensor("lse", [P, nt], f32).ap()
    wr = nc.alloc_sbuf_tensor("wr", [P, nt], f32).ap()
    wtmp = nc.alloc_sbuf_tensor("wtmp", [P, nt], f32).ap()
    outsb = nc.alloc_sbuf_tensor("outsb", [P, nt + NB], f32).ap()
    mscr = [nc.alloc_sbuf_tensor(f"mscr{k}", [P, P], f32).ap() for k in (0, 1)]
    warm = nc.alloc_sbuf_tensor("warm", [P, 1], f32).ap()

    psum = [nc.alloc_psum_tensor(f"psumblk{b}", [P, P], f32).ap()
            for b in range(NB)]

    # per-ring-slot DMA completion sems: same-queue DMAs can complete out
    # of order, so a shared counter cannot identify which tile landed
    def x_ready_wait(eng, i):
        eng.wait_ge(xslot[i % XB], 16 * (i // XB + 1))

    with (
        nc.Block() as block,
        ExitStack() as _sems,
        nc.semaphore("t_sem") as t_sem,
        nc.semaphore("band_sem") as band_sem,
        nc.semaphore("g_iota") as g_iota,
        nc.semaphore("h_sem") as h_sem,
        nc.semaphore("pe_tile") as pe_tile,
        nc.semaphore("lse_sem") as lse_sem,
        nc.semaphore("vec_final") as vec_final,
        nc.semaphore("odma") as odma,
        nc.semaphore("warm_sem") as warm_sem,
        nc.semaphore("act_sem") as act_sem,
        nc.semaphore("dve_sem") as dve_sem,
    ):
        xslot = [_sems.enter_context(nc.semaphore(f"xs{s}")) for s in range(XB)]

        # engines pipeline instruction issue: same-engine RAW/WAW needs
        # explicit completion waits (act_sem / dve_sem), like Tile emits.
        # x-slot recycling is derived from consumer completions:
        #   scalar consumed tile j  <=>  act_sem >= j + 2  (warm is +1)
        #   vector consumed tile j  <=>  h_sem   >= j + 1  (H after cast)
        def x_issue_guard(eng, i):
            if i >= XB:
                eng.wait_ge(act_sem, i - XB + 2)
                eng.wait_ge(h_sem, i - XB + 1)

        @block.sync
        def _(sync):
            for i in range(nt):
                x_issue_guard(sync, i)
                sync.dma_start(
                    out=x_ring[:, i % XB, :], in_=pred[i * P:(i + 1) * P, :]
                ).then_inc(xslot[i % XB], 16)
            sync.wait_ge(vec_final, 1)
            sync.dma_start(out=out, in_=outsb).then_inc(odma, 16)
            sync.wait_ge(odma, 16)

        @block.gpsimd
        def _(gpsimd):
            gpsimd.dma_start(out=t_pair, in_=tgt).then_inc(t_sem, 16)
            gpsimd.iota(iota_i, pattern=[[1, C]], base=0,
                        channel_multiplier=0).then_inc(g_iota, 1)
            gpsimd.dma_start(out=band_sb, in_=band).then_inc(band_sem, 16)

        @block.scalar
        def _(scalar):
            # warm the Exp/Ln table set before data arrives
            # Ln first => walrus loads the combined exp+ln table set once
            scalar.wait_ge(warm_sem, 1)
            scalar.activation(out=warm, in_=warm, func=AF.Ln).then_inc(
                act_sem, 1)
            for i in range(nt):
                # esc is single-buffered: allow at most 2 exps in flight
                # (act accumulator is pipelined 2-deep, same as Tile)
                if i >= 2:
                    scalar.wait_ge(act_sem, i)
                x_ready_wait(scalar, i)
                scalar.activation(
                    out=esc, in_=x_ring[:, i % XB, :], func=AF.Exp,
                    accum_out=sumexp[:, i:i + 1],
                ).then_inc(act_sem, 1)
            scalar.wait_ge(act_sem, nt + 1)  # every accumulate landed
            scalar.activation(out=lse, in_=sumexp, func=AF.Ln).then_inc(
                lse_sem, 1)

        @block.vector
        def _(vector):
            ndve = 0

            def dv(ins):
                nonlocal ndve
                ndve += 1
                return ins.then_inc(dve_sem, 1)

            vector.memset(warm, 1.0).then_inc(warm_sem, 1)
            vector.wait_ge(g_iota, 1)
            dv(vector.tensor_copy(out=iota_h, in_=iota_i))
            vector.wait_ge(t_sem, 16)
            dv(vector.tensor_copy(out=t_f, in_=t_pair[:, :, 0]))
            dv(vector.memset(wr, 1.0))
            for d in (1, 2, 3):
                vector.wait_ge(dve_sem, ndve)
                dv(vector.tensor_scalar(
                    out=wtmp, in0=t_f, scalar1=d - 0.5, scalar2=WDEC[d],
                    op0=OP.is_ge, op1=OP.mult))
                vector.wait_ge(dve_sem, ndve)
                dv(vector.tensor_tensor(out=wr, in0=wr, in1=wtmp, op=OP.add))
                vector.wait_ge(dve_sem, ndve)
                dv(vector.tensor_scalar(
                    out=wtmp, in0=t_f, scalar1=(C - 1 - d) + 0.5,
                    scalar2=WDEC[d], op0=OP.is_le, op1=OP.mult))
                vector.wait_ge(dve_sem, ndve)
                dv(vector.tensor_tensor(out=wr, in0=wr, in1=wtmp, op=OP.add))
            # preamble fully retired before the loop reads iota_h/t_f
            vector.wait_ge(dve_sem, ndve)
            for i in range(nt):
                if i >= HR:
                    vector.wait_ge(pe_tile, i - HR + 1)
                x_ready_wait(vector, i)
                dv(vector.tensor_copy(
                    out=xh_ring[:, i % HR, :], in_=x_ring[:, i % XB, :]))
                vector.tensor_scalar(
                    out=h_ring[:, i % HR, :], in0=iota_h,
                    scalar1=t_f[:, i:i + 1], scalar2=None,
                    op0=OP.is_equal, op1=OP.bypass,
                ).then_inc(h_sem, 1)
                ndve += 1  # the is_eq counts for ordering via h_sem instead
            vector.wait_ge(pe_tile, nt)
            vector.wait_ge(band_sem, 16)
            red_after = []
            for b in range(NB):
                # WAR guard: reduce for block b-2 must be done before
                # reusing its mscr buffer
                if b >= 2:
                    vector.wait_ge(dve_sem, red_after[b - 2])
                dv(vector.tensor_tensor(
                    out=mscr[b % 2], in0=psum[b],
                    in1=band_sb[:, b * P:(b + 1) * P], op=OP.mult))
                vector.wait_ge(dve_sem, ndve - nt)  # tt_b complete
                dv(vector.tensor_reduce(
                    out=outsb[:, nt + b:nt + b + 1], in_=mscr[b % 2],
                    axis=mybir.AxisListType.X, op=OP.add))
                red_after.append(ndve - nt)
            vector.wait_ge(dve_sem, ndve - nt)
            vector.wait_ge(lse_sem, 1)
            vector.tensor_tensor(
                out=outsb[:, 0:nt], in0=wr, in1=lse, op=OP.mult
            ).then_inc(vec_final, 1)

        @block.tensor
        def _(pe):
            for i in range(nt):
                pe.wait_ge(h_sem, i + 1)
                for b in range(NB):
                    s = BLK[b]
                    mm = pe.matmul(
                        psum[b], h_ring[:, i % HR, s:s + P],
                        xh_ring[:, i % HR, s:s + P],
                        start=(i == 0), stop=(i == nt - 1),
                    )
                mm.then_inc(pe_tile, 1)

    nc.compile()
    return nc


def _shard_inputs(prediction: np.ndarray, target: np.ndarray, rpc: int, ncores: int):
    pred = np.ascontiguousarray(np.asarray(prediction, dtype=np.float32)).reshape(-1, C)
    tgt = np.ascontiguousarray(np.asarray(target)).reshape(-1)
    assert tgt.dtype == np.int64
    tgt_pair = tgt.view(np.int32).reshape(-1, 2)  # little-endian: [:, 0] = low word
    nt = rpc // P
    band = _band_masks()
    in_maps = []
    for c in range(ncores):
        sl = slice(c * rpc, (c + 1) * rpc)
        in_maps.append({
            "prediction": pred[sl],
            "target_pair": np.ascontiguousarray(
                tgt_pair[sl].reshape(nt, P, 2).transpose(1, 0, 2)),
            "band": band,
        })
    return in_maps


def _host_combine(results, nt: int) -> np.float32:
    tot = 0.0
    nrows = 0
    for r in results:
        o = np.asarray(r["out"], dtype=np.float64)
        tot += o[:, :nt].sum() - o[:, nt:nt + NB].sum()
        nrows += P * nt
    return np.float32(tot / nrows)


def kernel(prediction: np.ndarray, target: np.ndarray, _trace: bool = False):
    nc = _build_raw(RPC)
    in_maps = _shard_inputs(prediction, target, RPC, NCORES)
    res = run_bass_kernel_spmd(
        nc, in_maps, core_ids=list(range(NCORES)), trace=_trace
    )
    loss = _host_combine(res.results, RPC // P)
    if _trace:
        return loss, res
    return loss


# revision 23
# speedup vs baseline: 1.0700x; 1.0700x over previous
"""CrossEntropyLossWithGaussianSmoothedLabels on 8 TRN2 NeuronCores.

Math: the reference's scatter-built smoothed label at class j is exactly
w[|j-t|] for |j-t|<=3 (w = [1, e^-.5, e^-1, e^-2]), clamped writes always
being overwritten by the nearer-distance write. So

  loss = mean_r( W_r * logsumexp(x_r) - sum_o w[|o|] * x_r[t_r+o] )

with W_r = sum of valid window weights. The gather term is computed on the
TensorEngine without any per-row gather:

  sum_r sum_o w[o] x[r, t_r+o] = sum_{|m-n|<=3} (H^T X)[m, n] * w[n-m]

where H is the one-hot target matrix. H^T X is accumulated in PSUM via 6
banded 128x128 fp16 matmuls per 128-row tile (overlapping class blocks with
ownership-deduped band masks). logsumexp runs max-free (|x| < 6) via the
ScalarEngine's fused exp+accumulate. Each core emits per-row W*lse and 6
per-partition band partials; the host sums (the unshard step) in float64.
"""

import math
from contextlib import ExitStack

import numpy as np

import concourse.bacc as bacc
import concourse.tile as tile
from concourse import mybir
from concourse.bass_utils import run_bass_kernel_spmd

P = 128
C = 722
NCORES = 8
ROWS = 16 * 2048
RPC = ROWS // NCORES  # 4096 rows per core
KPM = 4               # row-tiles per macro tile (per DMA)
NB = 6
BLK = [0, 124, 248, 372, 496, 594]  # even bases -> 4B-aligned fp16 slices
URANGES = [(0, 124), (124, 248), (248, 372), (372, 496), (496, 594), (594, 722)]
WDEC = [1.0, math.exp(-0.5), math.exp(-1.0), math.exp(-2.0)]

f32 = mybir.dt.float32
f16 = mybir.dt.float16
i32 = mybir.dt.int32


def _band_masks() -> np.ndarray:
    """[128, 6*128] f32: block-local band weights, each global band entry
    owned by exactly one block (by min(m,n) ownership range)."""
    m = np.zeros((P, NB * P), np.float32)
    for b in range(NB):
        s = BLK[b]
        lo, hi = URANGES[b]
        for i in range(P):
            for o in range(-3, 4):
                j = i + o
                if 0 <= j < P:
                    mg, ng = s + i, s + j
                    if mg < C and ng < C and lo <= min(mg, ng) < hi:
                        m[i, b * P + j] = WDEC[abs(o)]
    return m


def _build(rpc: int, stage: int = 6):
    nt = rpc // P
    nm = nt // KPM
    assert nt % KPM == 0
    nc = bacc.Bacc(
        "TRN2", target_bir_lowering=False, debug=False, num_devices=NCORES
    )
    AF = mybir.ActivationFunctionType
    OP = mybir.AluOpType

    pred = nc.dram_tensor("prediction", [rpc, C], f32, kind="ExternalInput").ap()
    tgt = nc.dram_tensor("target_pair", [P, rpc // P, 2], i32, kind="ExternalInput").ap()
    band = nc.dram_tensor("band", [P, NB * P], f32, kind="ExternalInput").ap()
    out = nc.dram_tensor("out", [P, nt + NB], f32, kind="ExternalOutput").ap()

    with tile.TileContext(nc) as tc, ExitStack() as ctx:
        xp = ctx.enter_context(tc.tile_pool(name="x", bufs=5))
        hp = ctx.enter_context(tc.tile_pool(name="h", bufs=4))
        cp = ctx.enter_context(tc.tile_pool(name="xc", bufs=4))
        sp = ctx.enter_context(tc.tile_pool(name="scr", bufs=2))
        sg = ctx.enter_context(tc.tile_pool(name="singles", bufs=1))
        pp = ctx.enter_context(tc.tile_pool(name="psum", bufs=1, space="PSUM"))

        # warm the Exp/Ln ACT table set at t~0 so the first real exp
        # doesn't eat the ~2.6us table load
        warm = sg.tile([P, 1], f32)
        nc.vector.memset(warm[:], 0.0)
        nc.scalar.activation(out=warm[:], in_=warm[:], func=AF.Exp)

        # row r lives at partition r // nt, column r % nt: every DMA line is
        # per-partition contiguous (target: 64 i32; prediction: 2888B rows)
        t_pair = sg.tile([P, nt, 2], i32)
        nc.sync.dma_start(out=t_pair[:], in_=tgt)

        # macro-batched loads: KPM row tiles per DMA, alternating queues
        xmac = {}

        def load_macro(m):
            xm = xp.tile([P, KPM, C], f32, name=f"xm{m}", tag="x")
            eng = nc.sync if m % 2 == 0 else nc.gpsimd
            src_ap = pred[m * KPM * P:(m + 1) * KPM * P, :].rearrange(
                "(k p) c -> p k c", p=P)
            eng.dma_start(out=xm[:], in_=src_ap)
            xmac[m] = xm

        nprefetch_mac = 3
        for m in range(min(nprefetch_mac, nt // KPM)):
            load_macro(m)

        band_sb = sg.tile([P, NB * P], f32)
        nc.sync.dma_start(out=band_sb[:], in_=band)
        outsb = sg.tile([P, nt + NB], f32)
        nc.vector.memset(outsb[:], 0.0)

        iota_i = sg.tile([P, C], i32)
        if stage >= 4:
            nc.gpsimd.iota(iota_i[:], pattern=[[1, C]], base=0, channel_multiplier=0)
        iota_h = sg.tile([P, C], f16)
        if stage >= 4:
            nc.vector.tensor_copy(out=iota_h[:], in_=iota_i[:])

        # t_f[p, i] = float(target[i*128 + p]) from the low int32 words
        t_f = sg.tile([P, nt], f32)
        if stage >= 3:
            nc.vector.tensor_copy(out=t_f[:], in_=t_pair[:, :, 0])

        sumexp = sg.tile([P, nt], f32)
        wr = sg.tile([P, nt], f32)

        psum_blk = [pp.tile([P, P], f32, name=f"psumblk{b}", tag=f"psum{b}") for b in range(NB)]

        # W_r = 1 + sum_d w_d*([t>=d] + [t<=721-d])
        nc.vector.memset(wr[:], 1.0)
        for d in (1, 2, 3) if stage >= 3 else ():
            tmp = sp.tile([P, nt], f32, tag="wtmp")
            nc.vector.tensor_scalar(
                out=tmp[:], in0=t_f[:], scalar1=d - 0.5, scalar2=WDEC[d],
                op0=OP.is_ge, op1=OP.mult,
            )
            nc.vector.tensor_tensor(out=wr[:], in0=wr[:], in1=tmp[:], op=OP.add)
            tmp2 = sp.tile([P, nt], f32, tag="wtmp")
            nc.vector.tensor_scalar(
                out=tmp2[:], in0=t_f[:], scalar1=(C - 1 - d) + 0.5, scalar2=WDEC[d],
                op0=OP.is_le, op1=OP.mult,
            )
            nc.vector.tensor_tensor(out=wr[:], in0=wr[:], in1=tmp2[:], op=OP.add)

        for i in range(nt) if stage >= 2 else ():
            m, k = divmod(i, KPM)
            if m not in xmac:
                load_macro(m)
            x = xmac[m][:, k, :]

            xh = cp.tile([P, C], f16, tag="xh")
            if stage >= 5:
                nc.vector.tensor_copy(out=xh[:], in_=x)

            esc = sp.tile([P, C], f16, tag="esc")
            nc.scalar.activation(
                out=esc[:], in_=x, func=AF.Exp,
                accum_out=sumexp[:, i:i + 1],
            )
            h = hp.tile([P, C], f16)
            if stage >= 4:
                nc.vector.tensor_scalar(
                    out=h[:], in0=iota_h[:], scalar1=t_f[:, i:i + 1],
                    scalar2=None, op0=OP.is_equal, op1=OP.bypass,
                )
            for b in range(NB) if stage >= 5 else ():
                s = BLK[b]
                nc.tensor.matmul(
                    psum_blk[b][:], h[:, s:s + P], xh[:, s:s + P],
                    start=(i == 0), stop=(i == nt - 1),
                )

        lse = sg.tile([P, nt], f32)
        if stage >= 2:
            nc.scalar.activation(out=lse[:], in_=sumexp[:], func=AF.Ln)
        if stage >= 3:
            nc.vector.tensor_tensor(out=outsb[:, 0:nt], in0=wr[:], in1=lse[:], op=OP.mult)
        elif stage >= 2:
            nc.vector.tensor_copy(out=outsb[:, 0:nt], in_=lse[:])
        for b in range(NB) if stage >= 6 else ():
            mscr = sp.tile([P, P], f32, tag="mscr")
            nc.vector.tensor_tensor(
                out=mscr[:], in0=psum_blk[b][:],
                in1=band_sb[:, b * P:(b + 1) * P], op=OP.mult,
            )
            nc.vector.tensor_reduce(
                out=outsb[:, nt + b:nt + b + 1], in_=mscr[:],
                axis=mybir.AxisListType.X, op=OP.add,
            )
        nc.sync.dma_start(out=out, in_=outsb[:])

    nc.compile()
    return nc


def _build_raw(rpc: int):
    """Raw-bacc version: explicit engine programs + semaphores, no Tile
    framework preamble/teardown."""
    nt = rpc // P
    nc = bacc.Bacc(
        "TRN2", target_bir_lowering=False, debug=False, num_devices=NCORES
    )
    AF = mybir.ActivationFunctionType
    OP = mybir.AluOpType

    pred = nc.dram_tensor("prediction", [rpc, C], f32, kind="ExternalInput").ap()
    tgt = nc.dram_tensor("target_pair", [P, nt, 2], i32, kind="ExternalInput").ap()
    band = nc.dram_tensor("band", [P, NB * P], f32, kind="ExternalInput").ap()
    out = nc.dram_tensor("out", [P, nt + NB], f32, kind="ExternalOutput").ap()

    XB = 14   # x ring depth
    HR = 4    # xh/h ring depth

    x_ring = nc.alloc_sbuf_tensor("x_ring", [P, XB, C], f32).ap()
    xh_ring = nc.alloc_sbuf_tensor("xh_ring", [P, HR, C], f16).ap()
    h_ring = nc.alloc_sbuf_tensor("h_ring", [P, HR, C], f16).ap()
    esc = nc.alloc_sbuf_tensor("esc", [P, C], f16).ap()
    iota_i = nc.alloc_sbuf_tensor("iota_i", [P, C], i32).ap()
    iota_h = nc.alloc_sbuf_tensor("iota_h", [P, C], f16).ap()
    t_pair = nc.alloc_sbuf_tensor("t_pair", [P, nt, 2], i32).ap()
    t_f = nc.alloc_sbuf_tensor("t_f", [P, nt], f32).ap()
    band_sb = nc.alloc_sbuf_tensor("band_sb", [P, NB * P], f32).ap()
    sumexp = nc.alloc_sbuf_tensor("sumexp", [P, nt], f32).ap()
    lse = nc.alloc_sbuf_tensor("lse", [P, nt], f32).ap()
    wr = nc.alloc_sbuf_tensor("wr", [P, nt], f32).ap()
    wtmp = nc.alloc_sbuf_tensor("wtmp", [P, nt], f32).ap()
    outsb = nc.alloc_sbuf_tensor("outsb", [P, nt + NB], f32).ap()
    mscr = [nc.alloc_sbuf_tensor(f"mscr{k}", [P, P], f32).ap() for k in (0, 1)]
    warm = nc.alloc_sbuf_tensor("warm", [P, 1], f32).ap()

    psum = [nc.alloc_psum_tensor(f"psumblk{b}", [P, P], f32).ap()
            for b in range(NB)]

    # per-ring-slot DMA completion sems: same-queue DMAs can complete out
    # of order, so a shared counter cannot identify which tile landed
    def x_ready_wait(eng, i):
        eng.wait_ge(xslot[i % XB], 16 * (i // XB + 1))

    with (
        nc.Block() as block,
        ExitStack() as _sems,
        nc.semaphore("t_sem") as t_sem,
        nc.semaphore("band_sem") as band_sem,
        nc.semaphore("g_iota") as g_iota,
        nc.semaphore("h_sem") as h_sem,
        nc.semaphore("pe_tile") as pe_tile,
        nc.semaphore("lse_sem") as lse_sem,
        nc.semaphore("vec_final") as vec_final,
        nc.semaphore("odma") as odma,
        nc.semaphore("warm_sem") as warm_sem,
        nc.semaphore("act_sem") as act_sem,
        nc.semaphore("dve_sem") as dve_sem,
    ):
        xslot = [_sems.enter_context(nc.semaphore(f"xs{s}")) for s in range(XB)]

        # engines pipeline instruction issue: same-engine RAW/WAW needs
        # explicit completion waits (act_sem / dve_sem), like Tile emits.
        # x-slot recycling is derived from consumer completions:
        #   scalar consumed tile j  <=>  act_sem >= j + 2  (warm is +1)
        #   vector consumed tile j  <=>  h_sem   >= j + 1  (H after cast)
        def x_issue_guard(eng, i):
            if i >= XB:
                eng.wait_ge(act_sem, i - XB + 2)
                eng.wait_ge(h_sem, i - XB + 1)

        @block.sync
        def _(sync):
            for i in range(nt):
                x_issue_guard(sync, i)
                sync.dma_start(
                    out=x_ring[:, i % XB, :], in_=pred[i * P:(i + 1) * P, :]
                ).then_inc(xslot[i % XB], 16)
            sync.wait_ge(vec_final, 1)
            sync.dma_start(out=out, in_=outsb).then_inc(odma, 16)
            sync.wait_ge(odma, 16)

        @block.gpsimd
        def _(gpsimd):
            gpsimd.dma_start(out=t_pair, in_=tgt).then_inc(t_sem, 16)
            gpsimd.iota(iota_i, pattern=[[1, C]], base=0,
                        channel_multiplier=0).then_inc(g_iota, 1)
            gpsimd.dma_start(out=band_sb, in_=band).then_inc(band_sem, 16)

        @block.scalar
        def _(scalar):
            # warm the Exp/Ln table set before data arrives
            scalar.wait_ge(warm_sem, 1)
            scalar.activation(out=warm, in_=warm, func=AF.Exp).then_inc(
                act_sem, 1)
            for i in range(nt):
                # esc is single-buffered: allow at most 2 exps in flight
                # (act accumulator is pipelined 2-deep, same as Tile)
                if i >= 2:
                    scalar.wait_ge(act_sem, i)
                x_ready_wait(scalar, i)
                scalar.activation(
                    out=esc, in_=x_ring[:, i % XB, :], func=AF.Exp,
                    accum_out=sumexp[:, i:i + 1],
                ).then_inc(act_sem, 1)
            scalar.wait_ge(act_sem, nt + 1)  # every accumulate landed
            scalar.activation(out=lse, in_=sumexp, func=AF.Ln).then_inc(
                lse_sem, 1)

        @block.vector
        def _(vector):
            ndve = 0

            def dv(ins):
                nonlocal ndve
                ndve += 1
                return ins.then_inc(dve_sem, 1)

            vector.memset(warm, 1.0).then_inc(warm_sem, 1)
            vector.wait_ge(g_iota, 1)
            dv(vector.tensor_copy(out=iota_h, in_=iota_i))
            vector.wait_ge(t_sem, 16)
            dv(vector.tensor_copy(out=t_f, in_=t_pair[:, :, 0]))
            dv(vector.memset(wr, 1.0))
            for d in (1, 2, 3):
                vector.wait_ge(dve_sem, ndve)
                dv(vector.tensor_scalar(
                    out=wtmp, in0=t_f, scalar1=d - 0.5, scalar2=WDEC[d],
                    op0=OP.is_ge, op1=OP.mult))
                vector.wait_ge(dve_sem, ndve)
                dv(vector.tensor_tensor(out=wr, in0=wr, in1=wtmp, op=OP.add))
                vector.wait_ge(dve_sem, ndve)
                dv(vector.tensor_scalar(
                    out=wtmp, in0=t_f, scalar1=(C - 1 - d) + 0.5,
                    scalar2=WDEC[d], op0=OP.is_le, op1=OP.mult))
                vector.wait_ge(dve_sem, ndve)
                dv(vector.tensor_tensor(out=wr, in0=wr, in1=wtmp, op=OP.add))
            # preamble fully retired before the loop reads iota_h/t_f
            vector.wait_ge(dve_sem, ndve)
            for i in range(nt):
                if i >= HR:
                    vector.wait_ge(pe_tile, i - HR + 1)
                x_ready_wait(vector, i)
                dv(vector.tensor_copy(
                    out=xh_ring[:, i % HR, :], in_=x_ring[:, i % XB, :]))
                vector.tensor_scalar(
                    out=h_ring[:, i % HR, :], in0=iota_h,
                    scalar1=t_f[:, i:i + 1], scalar2=None,
                    op0=OP.is_equal, op1=OP.bypass,
                ).then_inc(h_sem, 1)
                ndve += 1  # the is_eq counts for ordering via h_sem instead
            vector.wait_ge(pe_tile, nt)
            vector.wait_ge(band_sem, 16)
            red_after = []
            for b in range(NB):
                # WAR guard: reduce for block b-2 must be done before
                # reusing its mscr buffer
                if b >= 2:
                    vector.wait_ge(dve_sem, red_after[b - 2])
                dv(vector.tensor_tensor(
                    out=mscr[b % 2], in0=psum[b],
                    in1=band_sb[:, b * P:(b + 1) * P], op=OP.mult))
                vector.wait_ge(dve_sem, ndve - nt)  # tt_b complete
                dv(vector.tensor_reduce(
                    out=outsb[:, nt + b:nt + b + 1], in_=mscr[b % 2],
                    axis=mybir.AxisListType.X, op=OP.add))
                red_after.append(ndve - nt)
            vector.wait_ge(dve_sem, ndve - nt)
            vector.wait_ge(lse_sem, 1)
            vector.tensor_tensor(
                out=outsb[:, 0:nt], in0=wr, in1=lse, op=OP.mult
            ).then_inc(vec_final, 1)

        @block.tensor
        def _(pe):
            for i in range(nt):
                pe.wait_ge(h_sem, i + 1)
                for b in range(NB):
                    s = BLK[b]
                    mm = pe.matmul(
                        psum[b], h_ring[:, i % HR, s:s + P],
                        xh_ring[:, i % HR, s:s + P],
                        start=(i == 0), stop=(i == nt - 1),
                    )
                mm.then_inc(pe_tile, 1)

    nc.compile()
    return nc


def _shard_inputs(prediction: np.ndarray, target: np.ndarray, rpc: int, ncores: int):
    pred = np.ascontiguousarray(np.asarray(prediction, dtype=np.float32)).reshape(-1, C)
    tgt = np.ascontiguousarray(np.asarray(target)).reshape(-1)
    assert tgt.dtype == np.int64
    tgt_pair = tgt.view(np.int32).reshape(-1, 2)  # little-endian: [:, 0] = low word
    nt = rpc // P
    band = _band_masks()
    in_maps = []
    for c in range(ncores):
        sl = slice(c * rpc, (c + 1) * rpc)
        in_maps.append({
            "prediction": pred[sl],
            "target_pair": np.ascontiguousarray(
                tgt_pair[sl].reshape(nt, P, 2).transpose(1, 0, 2)),
            "band": band,
        })
    return in_maps


def _host_combine(results, nt: int) -> np.float32:
    tot = 0.0
    nrows = 0
    for r in results:
        o = np.asarray(r["out"], dtype=np.float64)
        tot += o[:, :nt].sum() - o[:, nt:nt + NB].sum()
        nrows += P * nt
    return np.float32(tot / nrows)


def kernel(prediction: np.ndarray, target: np.ndarray, _trace: bool = False):
    nc = _build_raw(RPC)
    in_maps = _shard_inputs(prediction, target, RPC, NCORES)
    res = run_bass_kernel_spmd(
        nc, in_maps, core_ids=list(range(NCORES)), trace=_trace
    )
    loss = _host_combine(res.results, RPC // P)
    if _trace:
        return loss, res
    return loss
